# revision 14
# baseline (speedup 1.0000x reference)
"""Trainium2 Bass kernel for nn_Attention_67637144977803.

Dense transformer attention block (XCiT-style, L2-normalized q/k along the
token axis), B=2, C=256, H=W=48 (N=2304 tokens), 8 heads x 64 dims.

Sharding: the 16 (batch, head) pairs are sharded 2-per-core across the 8
NeuronCores (cores 0-3: batch 0, cores 4-7: batch 1; core c%4 owns heads
2*(c%4), 2*(c%4)+1). Each core:
  1. computes its q/k/v slices via the 1x1-conv matmul (weights pre-sliced
     and pre-transposed on the host),
  2. l2-normalizes q, k along tokens,
  3. computes attention in the transposed layout S^T[m, n] = sum_d k[d,m]q[d,n]
     so softmax's contraction dim (m) lands on PSUM partitions,
  4. exp on the scalar engine (no max subtraction: normalized q/k make
     |S| < ~0.1, so exp is safely in range),
  5. AV matmul with a ones-row appended to v^T, which makes the softmax
     denominator fall out as row 64 of the PSUM accumulator,
  6. divides via reciprocal + a DMA round-trip through DRAM that broadcasts
     the reciprocal row across partitions, then a vector multiply,
  7. applies its slice of the output projection; the host sums the 4 partial
     projections per batch (bias is fed only to one core per batch).

All matmuls run as float32r (full-rate fp32 on the PE). f32r self-loading
matmuls only support a single sync-wait in the NEFF encoding, so the
dataflow is arranged so every matmul depends on at most one semaphore:
matmul inputs coming from DMA are staged through a DVE copy (which also
performs the required f32r rounding), v^T tiles are copied on the scalar
engine so AV matmuls only ever wait on the ACT counter, and the projection
matmul (which waits on DVE) keeps the PE's DVE clock fresh so PSUM-slot
release waits are already satisfied.
"""

import os
import sys

import numpy as np

for _p in ("/opt/trn_rl_repo", "/root/.axon_site/_ro/trn_rl_repo"):
    if os.path.isdir(_p) and _p not in sys.path:
        sys.path.insert(0, _p)

import concourse.bacc as bacc
import concourse.bass as bass
import concourse.mybir as mybir
import concourse.tile as tile
from concourse import bass_utils

F32 = mybir.dt.float32
F32R = mybir.dt.float32r

B = 2
C = 256
N = 2304  # 48*48 tokens
N_HEADS = 8
D = 64  # head dim
HEADS_PER_CORE = 2
N_CORES = 8
M_TILES = N // 128  # 18 contraction tiles over tokens
EXP_GROUP = 3  # QK psum banks per exp instruction
# token blocks (start, width); PSUM bank = 512 f32
BLOCKS = [(0, 512), (512, 512), (1024, 512), (1536, 512), (2048, 256)]

_CACHE = {}


def _build_kernel():
    """Build the (single-program SPMD) Bass module."""
    nc = bacc.Bacc("TRN2", target_bir_lowering=False, debug=False)

    x_d = nc.dram_tensor("x", [C, N], F32, kind="ExternalInput").ap()
    wq_d = nc.dram_tensor("wq", [C, 128], F32, kind="ExternalInput").ap()
    wk_d = nc.dram_tensor("wk", [C, 128], F32, kind="ExternalInput").ap()
    wv_d = nc.dram_tensor("wv", [C, 128], F32, kind="ExternalInput").ap()
    wp_d = nc.dram_tensor("wp", [128, C], F32, kind="ExternalInput").ap()
    ident_d = nc.dram_tensor("ident", [128, 128], F32, kind="ExternalInput").ap()
    ones_d = nc.dram_tensor("ones", [128, 1], F32, kind="ExternalInput").ap()
    bias_d = nc.dram_tensor("bias", [C, 1], F32, kind="ExternalInput").ap()
    y_d = nc.dram_tensor("y", [C, N], F32, kind="ExternalOutput").ap()

    with tile.TileContext(nc) as tc:
        _kernel_body(tc, x_d, wq_d, wk_d, wv_d, wp_d, ident_d, ones_d, bias_d, y_d)

    nc.compile()
    return nc


def _kernel_body(tc, x_d, wq_d, wk_d, wv_d, wp_d, ident_d, ones_d, bias_d, y_d):
    nc = tc.nc
    mult = mybir.AluOpType.mult
    add = mybir.AluOpType.add
    Exp = mybir.ActivationFunctionType.Exp

    from contextlib import ExitStack

    ctx = ExitStack()
    with ctx:
        const_pool = ctx.enter_context(tc.tile_pool(name="const", bufs=1))
        xw_pool = ctx.enter_context(tc.tile_pool(name="xw", bufs=1))
        qkv_pool = ctx.enter_context(tc.tile_pool(name="qkv", bufs=1))
        sexp_pool = ctx.enter_context(tc.tile_pool(name="sexp", bufs=2))
        small_pool = ctx.enter_context(tc.tile_pool(name="small", bufs=2))
        dram_pool = ctx.enter_context(tc.tile_pool(name="dscr", bufs=4, space="DRAM"))
        psum_s = ctx.enter_context(tc.tile_pool(name="ps", bufs=2, space="PSUM"))
        psum_av = ctx.enter_context(tc.tile_pool(name="pav", bufs=2, space="PSUM"))

        # ---- raw DMA loads (staged through DVE so matmuls wait on DVE only)
        x_raw = sexp_pool.tile([128, 2, N], F32, tag="sexp", name="x_raw")
        xv = x_d.rearrange("(a p) n -> p a n", p=128)
        nc.sync.dma_start(x_raw[:, 0], xv[:, 0])
        nc.sync.dma_start(x_raw[:, 1], xv[:, 1])

        w_raw = sexp_pool.tile([128, 3, 2, 128], F32, tag="sexp", name="w_raw")
        for wi, wd in enumerate((wq_d, wk_d, wv_d)):
            nc.sync.dma_start(w_raw[:, wi], wd.rearrange("(a p) m -> p a m", p=128))
        wp_raw = const_pool.tile([128, C], F32, name="wp_raw")
        nc.sync.dma_start(wp_raw[:], wp_d)
        ident_raw = const_pool.tile([128, 128], F32, name="ident_raw")
        nc.sync.dma_start(ident_raw[:], ident_d)
        ones_raw = const_pool.tile([128, 1], F32, name="ones_raw")
        nc.sync.dma_start(ones_raw[:], ones_d)
        bias_sb = const_pool.tile([128, 2], F32, name="bias_sb")
        nc.sync.dma_start(bias_sb[:], bias_d.rearrange("(a p) one -> p (a one)", p=128))

        # DVE staging copies (round to f32r where needed)
        x_sb = xw_pool.tile([128, 2, N], F32R, name="x_sb")
        nc.vector.tensor_copy(x_sb[:], x_raw[:])
        w_sb = xw_pool.tile([128, 3, 2, 128], F32R, name="w_sb")
        nc.vector.tensor_copy(w_sb[:], w_raw[:])
        wp_sb = xw_pool.tile([128, C], F32R, name="wp_sb")
        nc.vector.tensor_copy(wp_sb[:], wp_raw[:])
        ident = xw_pool.tile([128, 128], F32, name="ident")
        nc.vector.tensor_copy(ident[:], ident_raw[:])
        ones_col = xw_pool.tile([128, 1], F32R, name="ones_col")
        nc.vector.tensor_copy(ones_col[:], ones_raw[:])

        # ---- qkv projection: [128 rows = 2 heads x 64, N]
        q_sb = qkv_pool.tile([128, N], F32R, name="q_sb")
        k_sb = qkv_pool.tile([128, N], F32R, name="k_sb")
        v_sb = qkv_pool.tile([128, N], F32, name="v_sb")
        for wi, dst in ((0, q_sb), (1, k_sb), (2, v_sb)):
            for base, wdt in ((0, 1536), (1536, 768)):
                pt = psum_s.tile([128, 1536], F32, tag="ps", name=f"qkv_ps_{wi}_{base}")
                for j in range(0, wdt, 512):
                    w_ = min(512, wdt - j)
                    for kk in range(2):
                        nc.tensor.matmul(
                            pt[:, j : j + w_],
                            w_sb[:, wi, kk],
                            x_sb[:, kk, base + j : base + j + w_],
                            start=(kk == 0),
                            stop=(kk == 1),
                        )
                if wi == 2:
                    # v on DVE (transposes wait on the DVE counter)
                    nc.vector.tensor_copy(dst[:, base : base + wdt], pt[:, :wdt])
                else:
                    # q, k on ACT (idle during setup)
                    nc.scalar.copy(dst[:, base : base + wdt], pt[:, :wdt])

        # ---- l2-normalize q, k along tokens (free dim)
        for t_sb in (q_sb, k_sb):
            scr = sexp_pool.tile([128, N], F32, tag="sexp", name="norm_scr")
            ss = small_pool.tile([128, 1], F32, tag="ss", name="ss")
            nc.scalar.activation(
                scr[:, :N],
                t_sb[:],
                mybir.ActivationFunctionType.Square,
                accum_out=ss[:],
            )
            nrm = small_pool.tile([128, 1], F32, tag="nrm", name="nrm")
            nc.scalar.sqrt(nrm[:], ss[:])
            rn = small_pool.tile([128, 1], F32, tag="rn", name="rn")
            nc.vector.reciprocal(rn[:], nrm[:])
            nc.vector.tensor_scalar_mul(t_sb[:], t_sb[:], rn[:])

        # ---- v^T (+ ones row): [128 tokens-in-tile, (head, m-tile) x 65]
        vT = qkv_pool.tile([128, HEADS_PER_CORE * M_TILES * 65], F32R, name="vT")
        vT_v = vT.rearrange("p (t c) -> p t c", c=65)
        # ones row via ACT broadcast-copy (AV matmuls then only wait on ACT)
        nc.scalar.copy(
            vT_v[:, :, 64:65],
            ones_col[:].to_broadcast([128, HEADS_PER_CORE * M_TILES, 1]),
        )
        for h in range(HEADS_PER_CORE):
            for t in range(M_TILES):
                pt = psum_av.tile([128, 512], F32, tag="av", name=f"tr_{h}_{t}")
                nc.tensor.transpose(
                    pt[:, :64],
                    v_sb[h * 64 : (h + 1) * 64, t * 128 : (t + 1) * 128],
                    # diagonal block => 64x64 identity at the same base partition
                    ident[h * 64 : (h + 1) * 64, h * 64 : (h + 1) * 64],
                )
                # on ACT so AV matmuls merge this wait with the exp wait
                nc.scalar.copy(vT_v[:, h * M_TILES + t, 0:64], pt[:, :64])

        # ---- attention + projection, pipelined per token block
        out_sb = qkv_pool.tile([128, N], F32R, name="out_sb")
        y_sb = qkv_pool.tile([128, 2, N], F32, name="y_sb")
        yv = y_d.rearrange("(a p) n -> p a n", p=128)

        for nb, w in BLOCKS:
            for h in range(HEADS_PER_CORE):
                qh = q_sb[h * 64 : (h + 1) * 64]
                kh = k_sb[h * 64 : (h + 1) * 64]
                s_exp = sexp_pool.tile([128, M_TILES * 512], F32R, tag="sexp",
                                       name=f"s_exp_{nb}_{h}")
                n_groups = M_TILES // EXP_GROUP
                for g in range(n_groups):
                    pt = psum_s.tile([128, 1536], F32, tag="ps", name=f"qk_{nb}_{h}_{g}")
                    for j in range(EXP_GROUP):
                        m = g * EXP_GROUP + j
                        nc.tensor.matmul(
                            pt[:, j * 512 : j * 512 + w],
                            kh[:, m * 128 : (m + 1) * 128],
                            qh[:, nb : nb + w],
                            start=True,
                            stop=True,
                        )
                    # exp(group) -> s_exp columns [g*3*w, (g+1)*3*w)
                    o = s_exp[:, g * EXP_GROUP * w : (g + 1) * EXP_GROUP * w]
                    if w == 512:
                        nc.scalar.activation(o, pt[:, : EXP_GROUP * 512], Exp)
                    else:
                        i3 = pt.rearrange("p (b c) -> p b c", c=512)[:, :EXP_GROUP, :w]
                        o3 = o.rearrange("p (b c) -> p b c", c=w)
                        nc.scalar.activation(o3, i3, Exp)

                # AV: accumulate [65, w]; row 64 = softmax denominator
                po = psum_av.tile([128, 512], F32, tag="av", name=f"av_{nb}_{h}")
                for m in range(M_TILES):
                    nc.tensor.matmul(
                        po[:65, :w],
                        vT_v[:, h * M_TILES + m, :],
                        s_exp[:, m * w : (m + 1) * w],
                        start=(m == 0),
                        stop=(m == M_TILES - 1),
                    )

                # divide rows 0..63 by row 64: reciprocal, DMA round-trip
                # through DRAM to broadcast across partitions, multiply.
                rd = small_pool.tile([1, 512], F32, tag="rd", name=f"rd_{nb}_{h}")
                nc.vector.reciprocal(rd[:, :w], po[64:65, :w])
                scr_d = dram_pool.tile([1, 512], F32, tag="dscr", name=f"dscr_{nb}_{h}")
                nc.sync.dma_start(scr_d[:, :w], rd[:, :w])
                bc = small_pool.tile([64, 512], F32, tag="bc", name=f"bc_{nb}_{h}")
                nc.sync.dma_start(bc[:, :w], scr_d[:1, :w].to_broadcast([64, w]))
                nc.vector.tensor_mul(
                    out_sb[h * 64 : (h + 1) * 64, nb : nb + w],
                    po[0:64, :w],
                    bc[:, :w],
                )

            # projection for this token block (contracts both heads' 128 rows)
            pj = psum_s.tile([128, 1536], F32, tag="ps", name=f"proj_{nb}")
            for m2 in range(2):
                nc.tensor.matmul(
                    pj[:, m2 * 512 : m2 * 512 + w],
                    wp_sb[:, m2 * 128 : (m2 + 1) * 128],
                    out_sb[:, nb : nb + w],
                    start=True,
                    stop=True,
                )
                nc.vector.tensor_scalar_add(
                    y_sb[:, m2, nb : nb + w],
                    pj[:, m2 * 512 : m2 * 512 + w],
                    bias_sb[:, m2 : m2 + 1],
                )
            nc.sync.dma_start(yv[:, :, nb : nb + w], y_sb[:, :, nb : nb + w])


def _get_nc():
    if "nc" not in _CACHE:
        _CACHE["nc"] = _build_kernel()
    return _CACHE["nc"]


def _round_f32r(a):
    """Round fp32 to fp32r (TF32-like: 11-bit mantissa, round-half-up on
    magnitude). The on-device DVE staging copies also round, but rounding on
    the host keeps host and device data bit-identical."""
    u = np.ascontiguousarray(a, dtype=np.float32).view(np.uint32)
    r = ((u.astype(np.uint64) + 0x800) & 0xFFFFF000).astype(np.uint32)
    return r.view(np.float32)


def _make_in_maps(x, w_qkv, w_proj, b_proj):
    x = np.ascontiguousarray(np.asarray(x, dtype=np.float32)).reshape(B, C, N)
    w_qkv = np.asarray(w_qkv, dtype=np.float32)
    w_proj = np.asarray(w_proj, dtype=np.float32)
    b_proj = np.asarray(b_proj, dtype=np.float32)
    ident = np.eye(128, dtype=np.float32)

    in_maps = []
    for core in range(N_CORES):
        b = core // 4
        hg = core % 4
        r = 128 * hg
        wq = np.ascontiguousarray(w_qkv[r : r + 128, :].T)  # [C, 128]
        wk = np.ascontiguousarray(w_qkv[512 + r : 512 + r + 128, :].T)
        wv = np.ascontiguousarray(w_qkv[1024 + r : 1024 + r + 128, :].T)
        wp = np.ascontiguousarray(w_proj[:, r : r + 128].T)  # [128, C]
        bias = (
            b_proj.reshape(C, 1)
            if hg == 0
            else np.zeros((C, 1), dtype=np.float32)
        )
        in_maps.append(
            {
                "x": np.ascontiguousarray(x[b]),
                "wq": wq,
                "wk": wk,
                "wv": wv,
                "wp": wp,
                "ident": ident,
                "ones": np.ones((128, 1), dtype=np.float32),
                "bias": np.ascontiguousarray(bias),
            }
        )
    return in_maps


def run_spmd(x, w_qkv, w_proj, b_proj, trace=False):
    """Run the SPMD kernel on cores 0-7; returns (y, BassKernelResults)."""
    nc = _get_nc()
    in_maps = _make_in_maps(x, w_qkv, w_proj, b_proj)
    res = bass_utils.run_bass_kernel_spmd(
        nc, in_maps, core_ids=list(range(N_CORES)), trace=trace
    )
    y = np.zeros((B, C, N), dtype=np.float32)
    for core in range(N_CORES):
        y[core // 4] += res.results[core]["y"]
    return y.reshape(B, C, 48, 48), res


def kernel(x, w_qkv, w_proj, b_proj):
    y, _ = run_spmd(x, w_qkv, w_proj, b_proj, trace=False)
    return y


# revision 18
# speedup vs baseline: 1.3247x; 1.3247x over previous
"""Trainium2 Bass kernel for nn_Attention_67637144977803.

Dense transformer attention block (XCiT-style, L2-normalized q/k along the
token axis), B=2, C=256, H=W=48 (N=2304 tokens), 8 heads x 64 dims.

Sharding: the 16 (batch, head) pairs are sharded 2-per-core across the 8
NeuronCores (cores 0-3: batch 0, cores 4-7: batch 1; core c%4 owns heads
2*(c%4), 2*(c%4)+1). Each core:
  1. computes its q/k/v slices via the 1x1-conv matmul (weights pre-sliced
     and pre-transposed on the host),
  2. l2-normalizes q, k along tokens,
  3. computes attention in the transposed layout S^T[m, n] = sum_d k[d,m]q[d,n]
     so softmax's contraction dim (m) lands on PSUM partitions,
  4. exp on the scalar engine (no max subtraction: normalized q/k make
     |S| < ~0.1, so exp is safely in range),
  5. AV matmul with a ones-row appended to v^T, which makes the softmax
     denominator fall out as row 64 of the PSUM accumulator,
  6. divides via reciprocal + ones-matmul partition-broadcast + multiply,
  7. applies its slice of the output projection; the host sums the 4 partial
     projections per batch (bias is fed only to one core per batch).

All big matmuls run as float32r (full-rate fp32 on the PE); producers of
f32r-consumed data emit f32r so the BIR verifier's rounding rule holds
(DMA'd inputs are staged through a rounding DVE copy).

The (block, head) work items are software-pipelined: item i's QK+exp is
emitted BEFORE item i-1's AV+divide, so the scalar engine (the bottleneck:
~10.6M exp elements per core) never starves while the PE drains the
previous item's AV accumulation and projection.
"""

import os
import sys

import numpy as np

for _p in ("/opt/trn_rl_repo", "/root/.axon_site/_ro/trn_rl_repo"):
    if os.path.isdir(_p) and _p not in sys.path:
        sys.path.insert(0, _p)

import concourse.bacc as bacc
import concourse.bass as bass
import concourse.mybir as mybir
import concourse.tile as tile
from concourse import bass_utils

F32 = mybir.dt.float32
F32R = mybir.dt.float32r

B = 2
C = 256
N = 2304  # 48*48 tokens
N_HEADS = 8
D = 64  # head dim
HEADS_PER_CORE = 2
N_CORES = 8
M_TILES = N // 128  # 18 contraction tiles over tokens
EXP_GROUP = 3  # QK psum banks per exp instruction
# token blocks (start, width); PSUM bank = 512 f32
BLOCKS = [(0, 512), (512, 512), (1024, 512), (1536, 512), (2048, 256)]

# norms via gpsimd (Pool) square+reduce; fallback to ACT if compile rejects
NORM_ON_POOL = False

_CACHE = {}


def _build_kernel():
    """Build the (single-program SPMD) Bass module."""
    nc = bacc.Bacc("TRN2", target_bir_lowering=False, debug=False)

    x_d = nc.dram_tensor("x", [C, N], F32, kind="ExternalInput").ap()
    wq_d = nc.dram_tensor("wq", [C, 128], F32, kind="ExternalInput").ap()
    wk_d = nc.dram_tensor("wk", [C, 128], F32, kind="ExternalInput").ap()
    wv_d = nc.dram_tensor("wv", [C, 128], F32, kind="ExternalInput").ap()
    wp_d = nc.dram_tensor("wp", [128, C], F32, kind="ExternalInput").ap()
    ident_d = nc.dram_tensor("ident", [128, 128], F32, kind="ExternalInput").ap()
    ones_d = nc.dram_tensor("ones", [128, 64], F32, kind="ExternalInput").ap()
    bias_d = nc.dram_tensor("bias", [C, 1], F32, kind="ExternalInput").ap()
    y_d = nc.dram_tensor("y", [C, N], F32, kind="ExternalOutput").ap()

    with tile.TileContext(nc) as tc:
        _kernel_body(tc, x_d, wq_d, wk_d, wv_d, wp_d, ident_d, ones_d, bias_d, y_d)

    nc.compile()
    return nc


def _kernel_body(tc, x_d, wq_d, wk_d, wv_d, wp_d, ident_d, ones_d, bias_d, y_d):
    nc = tc.nc
    Exp = mybir.ActivationFunctionType.Exp

    from contextlib import ExitStack

    ctx = ExitStack()
    with ctx:
        const_pool = ctx.enter_context(tc.tile_pool(name="const", bufs=1))
        xw_pool = ctx.enter_context(tc.tile_pool(name="xw", bufs=1))
        qkv_pool = ctx.enter_context(tc.tile_pool(name="qkv", bufs=1))
        sexp_pool = ctx.enter_context(tc.tile_pool(name="sexp", bufs=2))
        small_pool = ctx.enter_context(tc.tile_pool(name="small", bufs=2))
        psum_s = ctx.enter_context(tc.tile_pool(name="ps", bufs=2, space="PSUM"))
        psum_av = ctx.enter_context(tc.tile_pool(name="pav", bufs=2, space="PSUM"))

        # ---- raw DMA loads
        x_raw = sexp_pool.tile([128, 2, N], F32, tag="sexp", name="x_raw")
        xv = x_d.rearrange("(a p) n -> p a n", p=128)
        w_raw = sexp_pool.tile([128, 3, 2, 128], F32, tag="sexp", name="w_raw")
        for wi, wd in enumerate((wq_d, wk_d, wv_d)):
            nc.sync.dma_start(w_raw[:, wi], wd.rearrange("(a p) m -> p a m", p=128))
        wp_raw = const_pool.tile([128, C], F32, name="wp_raw")
        nc.sync.dma_start(wp_raw[:], wp_d)
        ident_sb = const_pool.tile([128, 128], F32, name="ident_sb")
        nc.sync.dma_start(ident_sb[:], ident_d)
        ones_sb = const_pool.tile([128, 64], F32, name="ones_sb")
        nc.sync.dma_start(ones_sb[:], ones_d)
        bias_sb = const_pool.tile([128, 2], F32, name="bias_sb")
        nc.sync.dma_start(bias_sb[:], bias_d.rearrange("(a p) one -> p (a one)", p=128))

        # staged (f32r-rounded) copies; x chunked so qkv can start early
        x_sb = xw_pool.tile([128, 2, N], F32R, name="x_sb")
        for kk in range(2):
            for base, wdt in ((0, 1536), (1536, 768)):
                nc.sync.dma_start(
                    x_raw[:, kk, base : base + wdt], xv[:, kk, base : base + wdt]
                )
                nc.vector.tensor_copy(
                    x_sb[:, kk, base : base + wdt], x_raw[:, kk, base : base + wdt]
                )
        w_sb = xw_pool.tile([128, 3, 2, 128], F32R, name="w_sb")
        nc.vector.tensor_copy(w_sb[:], w_raw[:])
        wp_sb = xw_pool.tile([128, C], F32R, name="wp_sb")
        nc.vector.tensor_copy(wp_sb[:], wp_raw[:])
        ones_col = xw_pool.tile([128, 1], F32R, name="ones_col")
        nc.vector.tensor_copy(ones_col[:], ones_sb[:, 0:1])

        # ---- qkv projection: [128 rows = 2 heads x 64, N]; v first so the
        # transposes can begin while q/k are still being produced.
        q_sb = qkv_pool.tile([128, N], F32R, name="q_sb")
        k_sb = qkv_pool.tile([128, N], F32R, name="k_sb")
        v_sb = qkv_pool.tile([128, N], F32, name="v_sb")
        for wi, dst in ((2, v_sb), (0, q_sb), (1, k_sb)):
            for base, wdt in ((0, 1536), (1536, 768)):
                pt = psum_s.tile([128, 1536], F32, tag="ps", name=f"qkv_ps_{wi}_{base}")
                for j in range(0, wdt, 512):
                    w_ = min(512, wdt - j)
                    for kk in range(2):
                        nc.tensor.matmul(
                            pt[:, j : j + w_],
                            w_sb[:, wi, kk],
                            x_sb[:, kk, base + j : base + j + w_],
                            start=(kk == 0),
                            stop=(kk == 1),
                        )
                nc.vector.tensor_copy(dst[:, base : base + wdt], pt[:, :wdt])

        # ---- v^T (+ ones row): [128 tokens-in-tile, (head, m-tile) x 65]
        # transposes batched 8 per PSUM bank; one ACT copy per bank.
        vT = qkv_pool.tile([128, HEADS_PER_CORE * M_TILES * 65], F32R, name="vT")
        vT_v = vT.rearrange("p (t c) -> p t c", c=65)
        nc.scalar.copy(
            vT_v[:, :, 64:65],
            ones_col[:].to_broadcast([128, HEADS_PER_CORE * M_TILES, 1]),
        )
        n_vt = HEADS_PER_CORE * M_TILES  # 36
        for j in range(n_vt):
            h, t = divmod(j, M_TILES)
            pt = psum_av.tile([128, 512], F32, tag="av", name=f"tr_{j}")
            nc.tensor.matmul(
                pt[:, :64],
                v_sb[h * 64 : (h + 1) * 64, t * 128 : (t + 1) * 128],
                ident_sb[h * 64 : (h + 1) * 64, h * 64 : (h + 1) * 64],
                is_transpose=True,
                start=True,
                stop=True,
            )
            nc.scalar.copy(vT_v[:, j, 0:64], pt[:, :64])

        # ---- l2-normalize q, k along tokens (free dim)
        for t_sb in (q_sb, k_sb):
            scr = sexp_pool.tile([128, N], F32, tag="sexp", name="norm_scr")
            ss = small_pool.tile([128, 1], F32, tag="ss", name="ss")
            if NORM_ON_POOL:
                # square on the idle Pool engine; free-axis reduce is DVE-only
                nc.gpsimd.tensor_mul(scr[:, :N], t_sb[:], t_sb[:])
                nc.vector.reduce_sum(ss[:], scr[:, :N], axis=mybir.AxisListType.X)
            else:
                nc.scalar.activation(
                    scr[:, :N],
                    t_sb[:],
                    mybir.ActivationFunctionType.Square,
                    accum_out=ss[:],
                )
            nrm = small_pool.tile([128, 1], F32, tag="nrm", name="nrm")
            nc.scalar.sqrt(nrm[:], ss[:])
            rn = small_pool.tile([128, 1], F32, tag="rn", name="rn")
            nc.vector.reciprocal(rn[:], nrm[:])
            nc.vector.tensor_scalar_mul(t_sb[:], t_sb[:], rn[:])

        # ---- attention + projection, software-pipelined over (block, head)
        out_sb = qkv_pool.tile([128, N], F32R, name="out_sb")
        y_sb = qkv_pool.tile([128, 2, N], F32, name="y_sb")
        yv = y_d.rearrange("(a p) n -> p a n", p=128)

        def emit_qk_exp(nb, w, h):
            """QK matmuls + exp for one (block, head); returns s_exp tile."""
            qh = q_sb[h * 64 : (h + 1) * 64]
            kh = k_sb[h * 64 : (h + 1) * 64]
            s_exp = sexp_pool.tile(
                [128, M_TILES * 512], F32R, tag="sexp", name=f"s_exp_{nb}_{h}"
            )
            for g in range(M_TILES // EXP_GROUP):
                pt = psum_s.tile([128, 1536], F32, tag="ps", name=f"qk_{nb}_{h}_{g}")
                for j in range(EXP_GROUP):
                    m = g * EXP_GROUP + j
                    nc.tensor.matmul(
                        pt[:, j * 512 : j * 512 + w],
                        kh[:, m * 128 : (m + 1) * 128],
                        qh[:, nb : nb + w],
                        start=True,
                        stop=True,
                    )
                o = s_exp[:, g * EXP_GROUP * w : (g + 1) * EXP_GROUP * w]
                if w == 512:
                    nc.scalar.activation(o, pt[:, : EXP_GROUP * 512], Exp)
                else:
                    i3 = pt.rearrange("p (b c) -> p b c", c=512)[:, :EXP_GROUP, :w]
                    o3 = o.rearrange("p (b c) -> p b c", c=w)
                    nc.scalar.activation(o3, i3, Exp)
            return s_exp

        def emit_av_divide(nb, w, h, s_exp):
            """AV accumulation + softmax divide for one (block, head)."""
            po = psum_av.tile([128, 512], F32, tag="av", name=f"av_{nb}_{h}")
            for m in range(M_TILES):
                nc.tensor.matmul(
                    po[:65, :w],
                    vT_v[:, h * M_TILES + m, :],
                    s_exp[:, m * w : (m + 1) * w],
                    start=(m == 0),
                    stop=(m == M_TILES - 1),
                )
            rd = small_pool.tile([1, 512], F32, tag="rd", name=f"rd_{nb}_{h}")
            nc.vector.reciprocal(rd[:, :w], po[64:65, :w])
            # partition-broadcast of the reciprocal row via ones-matmul (f32)
            pb = psum_s.tile([128, 1536], F32, tag="ps", name=f"pb_{nb}_{h}")
            nc.tensor.matmul(
                pb[:64, :w], ones_sb[0:1, :], rd[:1, :w], start=True, stop=True
            )
            bc = small_pool.tile([64, 512], F32, tag="bc", name=f"bc_{nb}_{h}")
            nc.vector.tensor_copy(bc[:, :w], pb[:64, :w])
            nc.vector.tensor_mul(
                out_sb[h * 64 : (h + 1) * 64, nb : nb + w],
                po[0:64, :w],
                bc[:, :w],
            )

        def emit_proj(nb, w):
            """Output projection + bias + store for one token block."""
            pj = psum_s.tile([128, 1536], F32, tag="ps", name=f"proj_{nb}")
            for m2 in range(2):
                nc.tensor.matmul(
                    pj[:, m2 * 512 : m2 * 512 + w],
                    wp_sb[:, m2 * 128 : (m2 + 1) * 128],
                    out_sb[:, nb : nb + w],
                    start=True,
                    stop=True,
                )
                nc.vector.tensor_scalar_add(
                    y_sb[:, m2, nb : nb + w],
                    pj[:, m2 * 512 : m2 * 512 + w],
                    bias_sb[:, m2 : m2 + 1],
                )
            nc.sync.dma_start(yv[:, :, nb : nb + w], y_sb[:, :, nb : nb + w])

        items = [(nb, w, h) for (nb, w) in BLOCKS for h in range(HEADS_PER_CORE)]
        prev = None
        s_prev = None
        for it in items:
            s_cur = emit_qk_exp(*it)
            if prev is not None:
                emit_av_divide(*prev, s_prev)
                if prev[2] == HEADS_PER_CORE - 1:
                    emit_proj(prev[0], prev[1])
            prev, s_prev = it, s_cur
        emit_av_divide(*prev, s_prev)
        emit_proj(prev[0], prev[1])


def _get_nc():
    if "nc" not in _CACHE:
        _CACHE["nc"] = _build_kernel()
    return _CACHE["nc"]


def _round_f32r(a):
    """Round fp32 to fp32r (TF32-like: 11-bit mantissa, round-half-up on
    magnitude). The on-device DVE staging copies also round, but rounding on
    the host keeps host and device data bit-identical."""
    u = np.ascontiguousarray(a, dtype=np.float32).view(np.uint32)
    r = ((u.astype(np.uint64) + 0x800) & 0xFFFFF000).astype(np.uint32)
    return r.view(np.float32)


def _make_in_maps(x, w_qkv, w_proj, b_proj):
    x = np.ascontiguousarray(np.asarray(x, dtype=np.float32)).reshape(B, C, N)
    w_qkv = np.asarray(w_qkv, dtype=np.float32)
    w_proj = np.asarray(w_proj, dtype=np.float32)
    b_proj = np.asarray(b_proj, dtype=np.float32)
    ident = np.eye(128, dtype=np.float32)

    in_maps = []
    for core in range(N_CORES):
        b = core // 4
        hg = core % 4
        r = 128 * hg
        wq = np.ascontiguousarray(w_qkv[r : r + 128, :].T)  # [C, 128]
        wk = np.ascontiguousarray(w_qkv[512 + r : 512 + r + 128, :].T)
        wv = np.ascontiguousarray(w_qkv[1024 + r : 1024 + r + 128, :].T)
        wp = np.ascontiguousarray(w_proj[:, r : r + 128].T)  # [128, C]
        bias = (
            b_proj.reshape(C, 1)
            if hg == 0
            else np.zeros((C, 1), dtype=np.float32)
        )
        in_maps.append(
            {
                "x": np.ascontiguousarray(x[b]),
                "wq": wq,
                "wk": wk,
                "wv": wv,
                "wp": wp,
                "ident": ident,
                "ones": np.ones((128, 64), dtype=np.float32),
                "bias": np.ascontiguousarray(bias),
            }
        )
    return in_maps


def run_spmd(x, w_qkv, w_proj, b_proj, trace=False):
    """Run the SPMD kernel on cores 0-7; returns (y, BassKernelResults)."""
    nc = _get_nc()
    in_maps = _make_in_maps(x, w_qkv, w_proj, b_proj)
    res = bass_utils.run_bass_kernel_spmd(
        nc, in_maps, core_ids=list(range(N_CORES)), trace=trace
    )
    y = np.zeros((B, C, N), dtype=np.float32)
    for core in range(N_CORES):
        y[core // 4] += res.results[core]["y"]
    return y.reshape(B, C, 48, 48), res


def kernel(x, w_qkv, w_proj, b_proj):
    y, _ = run_spmd(x, w_qkv, w_proj, b_proj, trace=False)
    return y


# revision 19
# speedup vs baseline: 1.3929x; 1.0515x over previous
"""Trainium2 Bass kernel for nn_Attention_67637144977803.

Dense transformer attention block (XCiT-style, L2-normalized q/k along the
token axis), B=2, C=256, H=W=48 (N=2304 tokens), 8 heads x 64 dims.

Sharding: the 16 (batch, head) pairs are sharded 2-per-core across the 8
NeuronCores (cores 0-3: batch 0, cores 4-7: batch 1; core c%4 owns heads
2*(c%4), 2*(c%4)+1). Each core:
  1. computes its q/k/v slices via the 1x1-conv matmul (weights pre-sliced
     and pre-transposed on the host),
  2. l2-normalizes q, k along tokens,
  3. computes attention in the transposed layout S^T[m, n] = sum_d k[d,m]q[d,n]
     so softmax's contraction dim (m) lands on PSUM partitions,
  4. exp on the scalar engine (no max subtraction: normalized q/k make
     |S| < ~0.1, so exp is safely in range),
  5. AV matmul with a ones-row appended to v^T, which makes the softmax
     denominator fall out as row 64 of the PSUM accumulator,
  6. divides via reciprocal + ones-matmul partition-broadcast + multiply,
  7. applies its slice of the output projection; the host sums the 4 partial
     projections per batch (bias is fed only to one core per batch).

All big matmuls run as float32r (full-rate fp32 on the PE); producers of
f32r-consumed data emit f32r so the BIR verifier's rounding rule holds
(DMA'd inputs are staged through a rounding DVE copy).

The (block, head) work items are software-pipelined: item i's QK+exp is
emitted BEFORE item i-1's AV+divide, so the scalar engine (the bottleneck:
~10.6M exp elements per core) never starves while the PE drains the
previous item's AV accumulation and projection.
"""

import os
import sys

import numpy as np

for _p in ("/opt/trn_rl_repo", "/root/.axon_site/_ro/trn_rl_repo"):
    if os.path.isdir(_p) and _p not in sys.path:
        sys.path.insert(0, _p)

import concourse.bacc as bacc
import concourse.bass as bass
import concourse.mybir as mybir
import concourse.tile as tile
from concourse import bass_utils

F32 = mybir.dt.float32
F32R = mybir.dt.float32r

B = 2
C = 256
N = 2304  # 48*48 tokens
N_HEADS = 8
D = 64  # head dim
HEADS_PER_CORE = 2
N_CORES = 8
M_TILES = N // 128  # 18 contraction tiles over tokens
EXP_GROUP = 3  # QK psum banks per exp instruction
# token blocks (start, width); PSUM bank = 512 f32
BLOCKS = [(0, 512), (512, 512), (1024, 512), (1536, 512), (2048, 256)]

# norms via gpsimd (Pool) square+reduce; fallback to ACT if compile rejects
NORM_ON_POOL = True

_CACHE = {}


def _build_kernel():
    """Build the (single-program SPMD) Bass module."""
    nc = bacc.Bacc("TRN2", target_bir_lowering=False, debug=False)

    x_d = nc.dram_tensor("x", [C, N], F32, kind="ExternalInput").ap()
    wq_d = nc.dram_tensor("wq", [C, 128], F32, kind="ExternalInput").ap()
    wk_d = nc.dram_tensor("wk", [C, 128], F32, kind="ExternalInput").ap()
    wv_d = nc.dram_tensor("wv", [C, 128], F32, kind="ExternalInput").ap()
    wp_d = nc.dram_tensor("wp", [128, C], F32, kind="ExternalInput").ap()
    ident_d = nc.dram_tensor("ident", [128, 128], F32, kind="ExternalInput").ap()
    ones_d = nc.dram_tensor("ones", [128, 64], F32, kind="ExternalInput").ap()
    bias_d = nc.dram_tensor("bias", [C, 1], F32, kind="ExternalInput").ap()
    y_d = nc.dram_tensor("y", [C, N], F32, kind="ExternalOutput").ap()

    with tile.TileContext(nc) as tc:
        _kernel_body(tc, x_d, wq_d, wk_d, wv_d, wp_d, ident_d, ones_d, bias_d, y_d)

    nc.compile()
    return nc


def _kernel_body(tc, x_d, wq_d, wk_d, wv_d, wp_d, ident_d, ones_d, bias_d, y_d):
    nc = tc.nc
    Exp = mybir.ActivationFunctionType.Exp

    from contextlib import ExitStack

    ctx = ExitStack()
    with ctx:
        const_pool = ctx.enter_context(tc.tile_pool(name="const", bufs=1))
        xw_pool = ctx.enter_context(tc.tile_pool(name="xw", bufs=1))
        qkv_pool = ctx.enter_context(tc.tile_pool(name="qkv", bufs=1))
        sexp_pool = ctx.enter_context(tc.tile_pool(name="sexp", bufs=2))
        small_pool = ctx.enter_context(tc.tile_pool(name="small", bufs=2))
        dram_pool = ctx.enter_context(tc.tile_pool(name="dscr", bufs=4, space="DRAM"))
        psum_s = ctx.enter_context(tc.tile_pool(name="ps", bufs=2, space="PSUM"))
        psum_av = ctx.enter_context(tc.tile_pool(name="pav", bufs=2, space="PSUM"))

        # ---- raw DMA loads
        x_raw = sexp_pool.tile([128, 2, N], F32, tag="sexp", name="x_raw")
        xv = x_d.rearrange("(a p) n -> p a n", p=128)
        w_raw = sexp_pool.tile([128, 3, 2, 128], F32, tag="sexp", name="w_raw")
        for wi, wd in enumerate((wq_d, wk_d, wv_d)):
            nc.sync.dma_start(w_raw[:, wi], wd.rearrange("(a p) m -> p a m", p=128))
        wp_raw = const_pool.tile([128, C], F32, name="wp_raw")
        nc.sync.dma_start(wp_raw[:], wp_d)
        ident_sb = const_pool.tile([128, 128], F32, name="ident_sb")
        nc.sync.dma_start(ident_sb[:], ident_d)
        ones_sb = const_pool.tile([128, 64], F32, name="ones_sb")
        nc.sync.dma_start(ones_sb[:], ones_d)
        bias_sb = const_pool.tile([128, 2], F32, name="bias_sb")
        nc.sync.dma_start(bias_sb[:], bias_d.rearrange("(a p) one -> p (a one)", p=128))

        # staged (f32r-rounded) copies; x chunked so qkv can start early
        x_sb = xw_pool.tile([128, 2, N], F32R, name="x_sb")
        for kk in range(2):
            for base, wdt in ((0, 1536), (1536, 768)):
                nc.sync.dma_start(
                    x_raw[:, kk, base : base + wdt], xv[:, kk, base : base + wdt]
                )
                nc.vector.tensor_copy(
                    x_sb[:, kk, base : base + wdt], x_raw[:, kk, base : base + wdt]
                )
        w_sb = xw_pool.tile([128, 3, 2, 128], F32R, name="w_sb")
        nc.vector.tensor_copy(w_sb[:], w_raw[:])
        wp_sb = xw_pool.tile([128, C], F32R, name="wp_sb")
        nc.vector.tensor_copy(wp_sb[:], wp_raw[:])
        ones_col = xw_pool.tile([128, 1], F32R, name="ones_col")
        nc.vector.tensor_copy(ones_col[:], ones_sb[:, 0:1])

        # ---- qkv projection: [128 rows = 2 heads x 64, N]; v first so the
        # transposes can begin while q/k are still being produced.
        q_sb = qkv_pool.tile([128, N], F32R, name="q_sb")
        k_sb = qkv_pool.tile([128, N], F32R, name="k_sb")
        v_sb = qkv_pool.tile([128, N], F32, name="v_sb")
        for wi, dst in ((2, v_sb), (0, q_sb), (1, k_sb)):
            for base, wdt in ((0, 1536), (1536, 768)):
                pt = psum_s.tile([128, 1536], F32, tag="ps", name=f"qkv_ps_{wi}_{base}")
                for j in range(0, wdt, 512):
                    w_ = min(512, wdt - j)
                    for kk in range(2):
                        nc.tensor.matmul(
                            pt[:, j : j + w_],
                            w_sb[:, wi, kk],
                            x_sb[:, kk, base + j : base + j + w_],
                            start=(kk == 0),
                            stop=(kk == 1),
                        )
                if wi == 2:
                    nc.vector.tensor_copy(dst[:, base : base + wdt], pt[:, :wdt])
                else:
                    nc.scalar.copy(dst[:, base : base + wdt], pt[:, :wdt])

        # ---- v^T (+ ones row): [128 tokens-in-tile, (head, m-tile) x 65]
        # transposes batched 8 per PSUM bank; one ACT copy per bank.
        vT = qkv_pool.tile([128, HEADS_PER_CORE * M_TILES * 65], F32R, name="vT")
        vT_v = vT.rearrange("p (t c) -> p t c", c=65)
        nc.scalar.copy(
            vT_v[:, :, 64:65],
            ones_col[:].to_broadcast([128, HEADS_PER_CORE * M_TILES, 1]),
        )
        n_vt = HEADS_PER_CORE * M_TILES  # 36
        for j in range(n_vt):
            h, t = divmod(j, M_TILES)
            pt = psum_av.tile([128, 512], F32, tag="av", name=f"tr_{j}")
            nc.tensor.matmul(
                pt[:, :64],
                v_sb[h * 64 : (h + 1) * 64, t * 128 : (t + 1) * 128],
                ident_sb[h * 64 : (h + 1) * 64, h * 64 : (h + 1) * 64],
                is_transpose=True,
                start=True,
                stop=True,
            )
            nc.vector.tensor_copy(vT_v[:, j, 0:64], pt[:, :64])

        # ---- l2-normalize q, k along tokens (free dim)
        for t_sb in (q_sb, k_sb):
            scr = sexp_pool.tile([128, N], F32, tag="sexp", name="norm_scr")
            ss = small_pool.tile([128, 1], F32, tag="ss", name="ss")
            if NORM_ON_POOL:
                # square on the idle Pool engine; free-axis reduce is DVE-only
                nc.gpsimd.tensor_mul(scr[:, :N], t_sb[:], t_sb[:])
                nc.vector.reduce_sum(ss[:], scr[:, :N], axis=mybir.AxisListType.X)
            else:
                nc.scalar.activation(
                    scr[:, :N],
                    t_sb[:],
                    mybir.ActivationFunctionType.Square,
                    accum_out=ss[:],
                )
            nrm = small_pool.tile([128, 1], F32, tag="nrm", name="nrm")
            nc.scalar.sqrt(nrm[:], ss[:])
            rn = small_pool.tile([128, 1], F32, tag="rn", name="rn")
            nc.vector.reciprocal(rn[:], nrm[:])
            nc.vector.tensor_scalar_mul(t_sb[:], t_sb[:], rn[:])

        # ---- attention + projection, software-pipelined over (block, head)
        out_sb = qkv_pool.tile([128, N], F32R, name="out_sb")
        y_sb = qkv_pool.tile([128, 2, N], F32, name="y_sb")
        yv = y_d.rearrange("(a p) n -> p a n", p=128)

        def emit_qk_exp(nb, w, h):
            """QK matmuls + exp for one (block, head); returns s_exp tile."""
            qh = q_sb[h * 64 : (h + 1) * 64]
            kh = k_sb[h * 64 : (h + 1) * 64]
            s_exp = sexp_pool.tile(
                [128, M_TILES * 512], F32R, tag="sexp", name=f"s_exp_{nb}_{h}"
            )
            for g in range(M_TILES // EXP_GROUP):
                pt = psum_s.tile([128, 1536], F32, tag="ps", name=f"qk_{nb}_{h}_{g}")
                for j in range(EXP_GROUP):
                    m = g * EXP_GROUP + j
                    nc.tensor.matmul(
                        pt[:, j * 512 : j * 512 + w],
                        kh[:, m * 128 : (m + 1) * 128],
                        qh[:, nb : nb + w],
                        start=True,
                        stop=True,
                    )
                o = s_exp[:, g * EXP_GROUP * w : (g + 1) * EXP_GROUP * w]
                if w == 512:
                    nc.scalar.activation(o, pt[:, : EXP_GROUP * 512], Exp)
                else:
                    i3 = pt.rearrange("p (b c) -> p b c", c=512)[:, :EXP_GROUP, :w]
                    o3 = o.rearrange("p (b c) -> p b c", c=w)
                    nc.scalar.activation(o3, i3, Exp)
            return s_exp

        def emit_av_divide(nb, w, h, s_exp):
            """AV accumulation + softmax divide for one (block, head)."""
            po = psum_av.tile([128, 512], F32, tag="av", name=f"av_{nb}_{h}")
            for m in range(M_TILES):
                nc.tensor.matmul(
                    po[:65, :w],
                    vT_v[:, h * M_TILES + m, :],
                    s_exp[:, m * w : (m + 1) * w],
                    start=(m == 0),
                    stop=(m == M_TILES - 1),
                )
            rd = small_pool.tile([1, 512], F32, tag="rd", name=f"rd_{nb}_{h}")
            nc.vector.reciprocal(rd[:, :w], po[64:65, :w])
            # partition-broadcast via a DMA round-trip through DRAM
            scr_d = dram_pool.tile([1, 512], F32, tag="dscr", name=f"dscr_{nb}_{h}")
            nc.sync.dma_start(scr_d[:, :w], rd[:, :w])
            bc = small_pool.tile([64, 512], F32, tag="bc", name=f"bc_{nb}_{h}")
            nc.sync.dma_start(bc[:, :w], scr_d[:1, :w].to_broadcast([64, w]))
            nc.vector.tensor_mul(
                out_sb[h * 64 : (h + 1) * 64, nb : nb + w],
                po[0:64, :w],
                bc[:, :w],
            )

        def emit_proj(nb, w):
            """Output projection + bias + store for one token block."""
            pj = psum_s.tile([128, 1536], F32, tag="ps", name=f"proj_{nb}")
            for m2 in range(2):
                nc.tensor.matmul(
                    pj[:, m2 * 512 : m2 * 512 + w],
                    wp_sb[:, m2 * 128 : (m2 + 1) * 128],
                    out_sb[:, nb : nb + w],
                    start=True,
                    stop=True,
                )
                nc.vector.tensor_scalar_add(
                    y_sb[:, m2, nb : nb + w],
                    pj[:, m2 * 512 : m2 * 512 + w],
                    bias_sb[:, m2 : m2 + 1],
                )
            nc.sync.dma_start(yv[:, :, nb : nb + w], y_sb[:, :, nb : nb + w])

        items = [(nb, w, h) for (nb, w) in BLOCKS for h in range(HEADS_PER_CORE)]
        s_tiles = {}
        for idx, it in enumerate(items):
            s_tiles[idx] = emit_qk_exp(*it)
            if idx >= 1:
                pit = items[idx - 1]
                emit_av_divide(*pit, s_tiles.pop(idx - 1))
            if idx >= 2 and items[idx - 2][2] == HEADS_PER_CORE - 1:
                emit_proj(items[idx - 2][0], items[idx - 2][1])
        emit_av_divide(*items[-1], s_tiles.pop(len(items) - 1))
        if items[-2][2] == HEADS_PER_CORE - 1:
            emit_proj(items[-2][0], items[-2][1])
        emit_proj(items[-1][0], items[-1][1])


def _get_nc():
    if "nc" not in _CACHE:
        _CACHE["nc"] = _build_kernel()
    return _CACHE["nc"]


def _round_f32r(a):
    """Round fp32 to fp32r (TF32-like: 11-bit mantissa, round-half-up on
    magnitude). The on-device DVE staging copies also round, but rounding on
    the host keeps host and device data bit-identical."""
    u = np.ascontiguousarray(a, dtype=np.float32).view(np.uint32)
    r = ((u.astype(np.uint64) + 0x800) & 0xFFFFF000).astype(np.uint32)
    return r.view(np.float32)


def _make_in_maps(x, w_qkv, w_proj, b_proj):
    x = np.ascontiguousarray(np.asarray(x, dtype=np.float32)).reshape(B, C, N)
    w_qkv = np.asarray(w_qkv, dtype=np.float32)
    w_proj = np.asarray(w_proj, dtype=np.float32)
    b_proj = np.asarray(b_proj, dtype=np.float32)
    ident = np.eye(128, dtype=np.float32)

    in_maps = []
    for core in range(N_CORES):
        b = core // 4
        hg = core % 4
        r = 128 * hg
        wq = np.ascontiguousarray(w_qkv[r : r + 128, :].T)  # [C, 128]
        wk = np.ascontiguousarray(w_qkv[512 + r : 512 + r + 128, :].T)
        wv = np.ascontiguousarray(w_qkv[1024 + r : 1024 + r + 128, :].T)
        wp = np.ascontiguousarray(w_proj[:, r : r + 128].T)  # [128, C]
        bias = (
            b_proj.reshape(C, 1)
            if hg == 0
            else np.zeros((C, 1), dtype=np.float32)
        )
        in_maps.append(
            {
                "x": np.ascontiguousarray(x[b]),
                "wq": wq,
                "wk": wk,
                "wv": wv,
                "wp": wp,
                "ident": ident,
                "ones": np.ones((128, 64), dtype=np.float32),
                "bias": np.ascontiguousarray(bias),
            }
        )
    return in_maps


def run_spmd(x, w_qkv, w_proj, b_proj, trace=False):
    """Run the SPMD kernel on cores 0-7; returns (y, BassKernelResults)."""
    nc = _get_nc()
    in_maps = _make_in_maps(x, w_qkv, w_proj, b_proj)
    res = bass_utils.run_bass_kernel_spmd(
        nc, in_maps, core_ids=list(range(N_CORES)), trace=trace
    )
    y = np.zeros((B, C, N), dtype=np.float32)
    for core in range(N_CORES):
        y[core // 4] += res.results[core]["y"]
    return y.reshape(B, C, 48, 48), res


def kernel(x, w_qkv, w_proj, b_proj):
    y, _ = run_spmd(x, w_qkv, w_proj, b_proj, trace=False)
    return y


# revision 20
# speedup vs baseline: 1.4451x; 1.0375x over previous
"""Trainium2 Bass kernel for nn_Attention_67637144977803.

Dense transformer attention block (XCiT-style, L2-normalized q/k along the
token axis), B=2, C=256, H=W=48 (N=2304 tokens), 8 heads x 64 dims.

Sharding: the 16 (batch, head) pairs are sharded 2-per-core across the 8
NeuronCores (cores 0-3: batch 0, cores 4-7: batch 1; core c%4 owns heads
2*(c%4), 2*(c%4)+1). Each core:
  1. computes its q/k/v slices via the 1x1-conv matmul (weights pre-sliced
     and pre-transposed on the host),
  2. l2-normalizes q, k along tokens,
  3. computes attention in the transposed layout S^T[m, n] = sum_d k[d,m]q[d,n]
     so softmax's contraction dim (m) lands on PSUM partitions,
  4. exp on the scalar engine (no max subtraction: normalized q/k make
     |S| < ~0.1, so exp is safely in range),
  5. AV matmul with a ones-row appended to v^T, which makes the softmax
     denominator fall out as row 64 of the PSUM accumulator,
  6. divides via reciprocal + ones-matmul partition-broadcast + multiply,
  7. applies its slice of the output projection; the host sums the 4 partial
     projections per batch (bias is fed only to one core per batch).

All big matmuls run as float32r (full-rate fp32 on the PE); producers of
f32r-consumed data emit f32r so the BIR verifier's rounding rule holds
(DMA'd inputs are staged through a rounding DVE copy).

The (block, head) work items are software-pipelined: item i's QK+exp is
emitted BEFORE item i-1's AV+divide, so the scalar engine (the bottleneck:
~10.6M exp elements per core) never starves while the PE drains the
previous item's AV accumulation and projection.
"""

import os
import sys

import numpy as np

for _p in ("/opt/trn_rl_repo", "/root/.axon_site/_ro/trn_rl_repo"):
    if os.path.isdir(_p) and _p not in sys.path:
        sys.path.insert(0, _p)

import concourse.bacc as bacc
import concourse.bass as bass
import concourse.mybir as mybir
import concourse.tile as tile
from concourse import bass_utils

F32 = mybir.dt.float32
F32R = mybir.dt.float32r

B = 2
C = 256
N = 2304  # 48*48 tokens
N_HEADS = 8
D = 64  # head dim
HEADS_PER_CORE = 2
N_CORES = 8
M_TILES = N // 128  # 18 contraction tiles over tokens
EXP_GROUP = 3  # QK psum banks per exp instruction
# token blocks (start, width); PSUM bank = 512 f32
BLOCKS = [(0, 512), (512, 512), (1024, 512), (1536, 512), (2048, 256)]

# norms via gpsimd (Pool) square+reduce; fallback to ACT if compile rejects
NORM_ON_POOL = True

_CACHE = {}


def _build_kernel():
    """Build the (single-program SPMD) Bass module."""
    nc = bacc.Bacc("TRN2", target_bir_lowering=False, debug=False)

    x_d = nc.dram_tensor("x", [C, N], F32, kind="ExternalInput").ap()
    wq_d = nc.dram_tensor("wq", [C, 128], F32, kind="ExternalInput").ap()
    wk_d = nc.dram_tensor("wk", [C, 128], F32, kind="ExternalInput").ap()
    wv_d = nc.dram_tensor("wv", [C, 128], F32, kind="ExternalInput").ap()
    wp_d = nc.dram_tensor("wp", [128, C], F32, kind="ExternalInput").ap()
    ident_d = nc.dram_tensor("ident", [128, 128], F32, kind="ExternalInput").ap()
    ones_d = nc.dram_tensor("ones", [128, 64], F32, kind="ExternalInput").ap()
    bias_d = nc.dram_tensor("bias", [C, 1], F32, kind="ExternalInput").ap()
    y_d = nc.dram_tensor("y", [C, N], F32, kind="ExternalOutput").ap()

    with tile.TileContext(nc) as tc:
        _kernel_body(tc, x_d, wq_d, wk_d, wv_d, wp_d, ident_d, ones_d, bias_d, y_d)

    nc.compile()
    return nc


def _kernel_body(tc, x_d, wq_d, wk_d, wv_d, wp_d, ident_d, ones_d, bias_d, y_d):
    nc = tc.nc
    Exp = mybir.ActivationFunctionType.Exp

    from contextlib import ExitStack

    ctx = ExitStack()
    with ctx:
        const_pool = ctx.enter_context(tc.tile_pool(name="const", bufs=1))
        xw_pool = ctx.enter_context(tc.tile_pool(name="xw", bufs=1))
        qkv_pool = ctx.enter_context(tc.tile_pool(name="qkv", bufs=1))
        sexp_pool = ctx.enter_context(tc.tile_pool(name="sexp", bufs=2))
        small_pool = ctx.enter_context(tc.tile_pool(name="small", bufs=2))
        dram_pool = ctx.enter_context(tc.tile_pool(name="dscr", bufs=4, space="DRAM"))
        psum_s = ctx.enter_context(tc.tile_pool(name="ps", bufs=2, space="PSUM"))
        psum_av = ctx.enter_context(tc.tile_pool(name="pav", bufs=2, space="PSUM"))

        # ---- raw DMA loads: x chunk 0 (both c-tiles) first so qkv starts
        # as early as possible, then the weights, then constants.
        x_raw = sexp_pool.tile([128, 2, N], F32, tag="sexp", name="x_raw")
        xv = x_d.rearrange("(a p) n -> p a n", p=128)
        x_sb = xw_pool.tile([128, 2, N], F32R, name="x_sb")
        w_raw = sexp_pool.tile([128, 3, 2, 128], F32, tag="sexp", name="w_raw")
        for base, wdt in ((0, 1536), (1536, 768)):
            for kk in range(2):
                nc.sync.dma_start(
                    x_raw[:, kk, base : base + wdt], xv[:, kk, base : base + wdt]
                )
                nc.vector.tensor_copy(
                    x_sb[:, kk, base : base + wdt], x_raw[:, kk, base : base + wdt]
                )
            if base == 0:
                for wi, wd in enumerate((wq_d, wk_d, wv_d)):
                    nc.sync.dma_start(
                        w_raw[:, wi], wd.rearrange("(a p) m -> p a m", p=128)
                    )
        w_sb = xw_pool.tile([128, 3, 2, 128], F32R, name="w_sb")
        nc.vector.tensor_copy(w_sb[:], w_raw[:])
        wp_raw = const_pool.tile([128, C], F32, name="wp_raw")
        nc.sync.dma_start(wp_raw[:], wp_d)
        ident_sb = const_pool.tile([128, 128], F32, name="ident_sb")
        nc.sync.dma_start(ident_sb[:], ident_d)
        ones_sb = const_pool.tile([128, 64], F32, name="ones_sb")
        nc.sync.dma_start(ones_sb[:], ones_d)
        bias_sb = const_pool.tile([128, 2], F32, name="bias_sb")
        nc.sync.dma_start(bias_sb[:], bias_d.rearrange("(a p) one -> p (a one)", p=128))
        wp_sb = xw_pool.tile([128, C], F32R, name="wp_sb")
        nc.vector.tensor_copy(wp_sb[:], wp_raw[:])
        ones_col = xw_pool.tile([128, 1], F32R, name="ones_col")
        nc.vector.tensor_copy(ones_col[:], ones_sb[:, 0:1])
        # preload the exp activation table before it's on the critical path
        warm = small_pool.tile([128, 1], F32, tag="ss", name="warm")
        nc.scalar.activation(warm[:], ones_sb[:, 0:1], Exp)

        # ---- qkv projection: [128 rows = 2 heads x 64, N]; k and q first
        # (the QK critical path), v last (transposes overlap the first exps).
        # Norm partial sums are computed per chunk to overlap the chain.
        q_sb = qkv_pool.tile([128, N], F32R, name="q_sb")
        k_sb = qkv_pool.tile([128, N], F32R, name="k_sb")
        v_sb = qkv_pool.tile([128, N], F32, name="v_sb")
        ss_parts = {}
        for wi, dst in ((1, k_sb), (0, q_sb), (2, v_sb)):
            for ci, (base, wdt) in enumerate(((0, 1536), (1536, 768))):
                pt = psum_s.tile([128, 1536], F32, tag="ps", name=f"qkv_ps_{wi}_{base}")
                for j in range(0, wdt, 512):
                    w_ = min(512, wdt - j)
                    for kk in range(2):
                        nc.tensor.matmul(
                            pt[:, j : j + w_],
                            w_sb[:, wi, kk],
                            x_sb[:, kk, base + j : base + j + w_],
                            start=(kk == 0),
                            stop=(kk == 1),
                        )
                nc.vector.tensor_copy(dst[:, base : base + wdt], pt[:, :wdt])
                if wi != 2:
                    scr = sexp_pool.tile([128, N], F32, tag="sexp",
                                         name=f"sq_{wi}_{base}")
                    nc.gpsimd.tensor_mul(
                        scr[:, base : base + wdt],
                        dst[:, base : base + wdt],
                        dst[:, base : base + wdt],
                    )
                    ssp = small_pool.tile([128, 1], F32, tag=f"ssp{ci}",
                                          name=f"ssp_{wi}_{base}")
                    nc.vector.reduce_sum(
                        ssp[:], scr[:, base : base + wdt], axis=mybir.AxisListType.X
                    )
                    ss_parts[(wi, ci)] = ssp

        # ---- v^T (+ ones row): [128 tokens-in-tile, (head, m-tile) x 65]
        # transposes batched 8 per PSUM bank; one ACT copy per bank.
        vT = qkv_pool.tile([128, HEADS_PER_CORE * M_TILES * 65], F32R, name="vT")
        vT_v = vT.rearrange("p (t c) -> p t c", c=65)
        nc.vector.tensor_copy(
            vT_v[:, :, 64:65],
            ones_col[:].to_broadcast([128, HEADS_PER_CORE * M_TILES, 1]),
        )
        n_vt = HEADS_PER_CORE * M_TILES  # 36
        for j in range(n_vt):
            h, t = divmod(j, M_TILES)
            pt = psum_av.tile([128, 512], F32, tag="av", name=f"tr_{j}")
            nc.tensor.matmul(
                pt[:, :64],
                v_sb[h * 64 : (h + 1) * 64, t * 128 : (t + 1) * 128],
                ident_sb[h * 64 : (h + 1) * 64, h * 64 : (h + 1) * 64],
                is_transpose=True,
                start=True,
                stop=True,
            )
            nc.vector.tensor_copy(vT_v[:, j, 0:64], pt[:, :64])

        # ---- l2-normalize q, k along tokens: combine chunk partials
        for wi, t_sb in ((1, k_sb), (0, q_sb)):
            ss = small_pool.tile([128, 1], F32, tag="ss", name=f"ss_{wi}")
            nc.vector.tensor_add(ss[:], ss_parts[(wi, 0)][:], ss_parts[(wi, 1)][:])
            nrm = small_pool.tile([128, 1], F32, tag="nrm", name=f"nrm_{wi}")
            nc.scalar.sqrt(nrm[:], ss[:])
            rn = small_pool.tile([128, 1], F32, tag="rn", name=f"rn_{wi}")
            nc.vector.reciprocal(rn[:], nrm[:])
            # scale in two chunks so the first QK block can start early
            nc.vector.tensor_scalar_mul(t_sb[:, 0:512], t_sb[:, 0:512], rn[:])
            nc.vector.tensor_scalar_mul(t_sb[:, 512:N], t_sb[:, 512:N], rn[:])

        # ---- attention + projection, software-pipelined over (block, head)
        out_sb = qkv_pool.tile([128, N], F32R, name="out_sb")
        y_sb = qkv_pool.tile([128, 2, N], F32, name="y_sb")
        yv = y_d.rearrange("(a p) n -> p a n", p=128)

        def emit_qk_exp(nb, w, h):
            """QK matmuls + exp for one (block, head); returns s_exp tile."""
            qh = q_sb[h * 64 : (h + 1) * 64]
            kh = k_sb[h * 64 : (h + 1) * 64]
            s_exp = sexp_pool.tile(
                [128, M_TILES * 512], F32R, tag="sexp", name=f"s_exp_{nb}_{h}"
            )
            for g in range(M_TILES // EXP_GROUP):
                pt = psum_s.tile([128, 1536], F32, tag="ps", name=f"qk_{nb}_{h}_{g}")
                for j in range(EXP_GROUP):
                    m = g * EXP_GROUP + j
                    nc.tensor.matmul(
                        pt[:, j * 512 : j * 512 + w],
                        kh[:, m * 128 : (m + 1) * 128],
                        qh[:, nb : nb + w],
                        start=True,
                        stop=True,
                    )
                o = s_exp[:, g * EXP_GROUP * w : (g + 1) * EXP_GROUP * w]
                if w == 512:
                    nc.scalar.activation(o, pt[:, : EXP_GROUP * 512], Exp)
                else:
                    i3 = pt.rearrange("p (b c) -> p b c", c=512)[:, :EXP_GROUP, :w]
                    o3 = o.rearrange("p (b c) -> p b c", c=w)
                    nc.scalar.activation(o3, i3, Exp)
            return s_exp

        def emit_av_divide(nb, w, h, s_exp):
            """AV accumulation + softmax divide for one (block, head)."""
            po = psum_av.tile([128, 512], F32, tag="av", name=f"av_{nb}_{h}")
            for m in range(M_TILES):
                nc.tensor.matmul(
                    po[:65, :w],
                    vT_v[:, h * M_TILES + m, :],
                    s_exp[:, m * w : (m + 1) * w],
                    start=(m == 0),
                    stop=(m == M_TILES - 1),
                )
            rd = small_pool.tile([1, 512], F32, tag="rd", name=f"rd_{nb}_{h}")
            nc.vector.reciprocal(rd[:, :w], po[64:65, :w])
            # partition-broadcast via a DMA round-trip through DRAM
            scr_d = dram_pool.tile([1, 512], F32, tag="dscr", name=f"dscr_{nb}_{h}")
            nc.sync.dma_start(scr_d[:, :w], rd[:, :w])
            bc = small_pool.tile([64, 512], F32, tag="bc", name=f"bc_{nb}_{h}")
            nc.sync.dma_start(bc[:, :w], scr_d[:1, :w].to_broadcast([64, w]))
            nc.vector.tensor_mul(
                out_sb[h * 64 : (h + 1) * 64, nb : nb + w],
                po[0:64, :w],
                bc[:, :w],
            )

        def emit_proj(nb, w):
            """Output projection + bias + store for one token block."""
            pj = psum_s.tile([128, 1536], F32, tag="ps", name=f"proj_{nb}")
            for m2 in range(2):
                nc.tensor.matmul(
                    pj[:, m2 * 512 : m2 * 512 + w],
                    wp_sb[:, m2 * 128 : (m2 + 1) * 128],
                    out_sb[:, nb : nb + w],
                    start=True,
                    stop=True,
                )
                nc.vector.tensor_scalar_add(
                    y_sb[:, m2, nb : nb + w],
                    pj[:, m2 * 512 : m2 * 512 + w],
                    bias_sb[:, m2 : m2 + 1],
                )
            nc.sync.dma_start(yv[:, :, nb : nb + w], y_sb[:, :, nb : nb + w])

        items = [(nb, w, h) for (nb, w) in BLOCKS for h in range(HEADS_PER_CORE)]
        s_tiles = {}
        for idx, it in enumerate(items):
            s_tiles[idx] = emit_qk_exp(*it)
            if idx >= 1:
                pit = items[idx - 1]
                emit_av_divide(*pit, s_tiles.pop(idx - 1))
            if idx >= 2 and items[idx - 2][2] == HEADS_PER_CORE - 1:
                emit_proj(items[idx - 2][0], items[idx - 2][1])
        emit_av_divide(*items[-1], s_tiles.pop(len(items) - 1))
        if items[-2][2] == HEADS_PER_CORE - 1:
            emit_proj(items[-2][0], items[-2][1])
        emit_proj(items[-1][0], items[-1][1])


def _get_nc():
    if "nc" not in _CACHE:
        _CACHE["nc"] = _build_kernel()
    return _CACHE["nc"]


def _round_f32r(a):
    """Round fp32 to fp32r (TF32-like: 11-bit mantissa, round-half-up on
    magnitude). The on-device DVE staging copies also round, but rounding on
    the host keeps host and device data bit-identical."""
    u = np.ascontiguousarray(a, dtype=np.float32).view(np.uint32)
    r = ((u.astype(np.uint64) + 0x800) & 0xFFFFF000).astype(np.uint32)
    return r.view(np.float32)


def _make_in_maps(x, w_qkv, w_proj, b_proj):
    x = np.ascontiguousarray(np.asarray(x, dtype=np.float32)).reshape(B, C, N)
    w_qkv = np.asarray(w_qkv, dtype=np.float32)
    w_proj = np.asarray(w_proj, dtype=np.float32)
    b_proj = np.asarray(b_proj, dtype=np.float32)
    ident = np.eye(128, dtype=np.float32)

    in_maps = []
    for core in range(N_CORES):
        b = core // 4
        hg = core % 4
        r = 128 * hg
        wq = np.ascontiguousarray(w_qkv[r : r + 128, :].T)  # [C, 128]
        wk = np.ascontiguousarray(w_qkv[512 + r : 512 + r + 128, :].T)
        wv = np.ascontiguousarray(w_qkv[1024 + r : 1024 + r + 128, :].T)
        wp = np.ascontiguousarray(w_proj[:, r : r + 128].T)  # [128, C]
        bias = (
            b_proj.reshape(C, 1)
            if hg == 0
            else np.zeros((C, 1), dtype=np.float32)
        )
        in_maps.append(
            {
                "x": np.ascontiguousarray(x[b]),
                "wq": wq,
                "wk": wk,
                "wv": wv,
                "wp": wp,
                "ident": ident,
                "ones": np.ones((128, 64), dtype=np.float32),
                "bias": np.ascontiguousarray(bias),
            }
        )
    return in_maps


def run_spmd(x, w_qkv, w_proj, b_proj, trace=False):
    """Run the SPMD kernel on cores 0-7; returns (y, BassKernelResults)."""
    nc = _get_nc()
    in_maps = _make_in_maps(x, w_qkv, w_proj, b_proj)
    res = bass_utils.run_bass_kernel_spmd(
        nc, in_maps, core_ids=list(range(N_CORES)), trace=trace
    )
    y = np.zeros((B, C, N), dtype=np.float32)
    for core in range(N_CORES):
        y[core // 4] += res.results[core]["y"]
    return y.reshape(B, C, 48, 48), res


def kernel(x, w_qkv, w_proj, b_proj):
    y, _ = run_spmd(x, w_qkv, w_proj, b_proj, trace=False)
    return y
